# revision 20
# baseline (speedup 1.0000x reference)
"""Multi-head attention block (dense transformer) on 8 Trainium2 NeuronCores.

Problem: x [4, 2048, 1024] f32, w_qkv [1024, 3072], w_out [1024, 1024].
  qkv = x @ w_qkv -> split (3, 16 heads, 64) -> softmax(q k^T / 8) v -> @ w_out

Sharding: tensor-parallel over heads. Core c owns heads (2c, 2c+1):
  - w_qkv columns for q/k/v of those heads -> [1024, 384]
  - w_out rows for those heads            -> [128, 1024]
  - x is pre-transposed/cast on the host to xT [1024, 8192] bf16
  - each core computes a full-shape partial output [8192, 1024] bf16; the
    host f32 sum of the 8 partials is the all-reduce.

Per-core engine budget (measured): ACT exp of the 8 local attention
matrices = 256 x [128,1024] tiles ~ 285us; PE matmul work ~ 340us; DVE
evacuations ~ 200us. The kernel is PE/ACT co-bound, so the emission is a
fine-grained weave: every nj iteration of P2 carries its score pair, exp,
av pair, AND a ~2-matmul dribble of P1 (next batch's qkv projection +
v transposes) and P3 (previous batch's output projection), so neither PE
nor ACT ever sits behind a block of the other's work (PE queue is strict
FIFO - emission order is execution order).

Layouts:
  qT/kT [128, 2048] bf16: rows = [head A d 0:64 | head B d 64:128].
  scoresT pair: row-group packed (K=64 concurrent matmuls h0/h64).
  v_sb [128, nj, 136]: [1 | vA(64) | vB(64) | 1 | pad] - one contiguous
    [128,128] copy from the PE transpose; ones columns give softmax sums
    as row 0 (head A, ones-first) / row 64 (head B, ones-last) of the
    M=65 av matmuls.
  PSUM: work 2 banks (P1/P3/transpose ping-pong), scores 2x2 banks
    (ping-pong against ACT), av 2 banks (pavA/pavB per ni).
"""

import numpy as np
import ml_dtypes

import concourse.bacc as bacc
import concourse.tile as tile
from concourse import mybir, masks
from concourse.bass_utils import run_bass_kernel_spmd

F32 = mybir.dt.float32
BF16 = mybir.dt.bfloat16
EXP = mybir.ActivationFunctionType.Exp

B = 4
PRIO = 1000000
N = 2048
D = 1024
HEADS = 16
DH = 64
NT = B * N           # 8192 tokens
FT = D // 128        # 8 feature chunks
NI = 4               # n_i tiles of 512 per batch
NJ = 16              # n_j chunks of 128 per batch
VW = 136             # v chunk: [1 | vA(64) | vB(64) | 1 | pad6] (16B-aligned)

_CACHE = {}


def build():
    nc = bacc.Bacc("TRN2", target_bir_lowering=False, debug=False, num_devices=1)
    xT_d = nc.dram_tensor("xT", [D, NT], BF16, kind="ExternalInput").ap()
    wqkv_d = nc.dram_tensor("wqkv", [D, 384], BF16, kind="ExternalInput").ap()
    wout_d = nc.dram_tensor("wout", [128, D], BF16, kind="ExternalInput").ap()
    out_d = nc.dram_tensor("out", [NT, D], BF16, kind="ExternalOutput").ap()
    xT_v = xT_d.rearrange("(f p) n -> f p n", p=128)

    with tile.TileContext(nc) as tc:
        with tc.tile_pool(name="const", bufs=1) as cpool, \
             tc.tile_pool(name="xt", bufs=4) as xt_pool, \
             tc.tile_pool(name="qkv", bufs=2) as qkv_pool, \
             tc.tile_pool(name="vt", bufs=3) as vt_pool, \
             tc.tile_pool(name="attn", bufs=4) as attn_pool, \
             tc.tile_pool(name="ostk", bufs=2) as ostk_pool, \
             tc.tile_pool(name="ov", bufs=4) as ov_pool, \
             tc.tile_pool(name="smol", bufs=6) as smol_pool, \
             tc.tile_pool(name="fout", bufs=2) as fout_pool, \
             tc.tile_pool(name="ps_work", bufs=2, space="PSUM") as ps_work, \
             tc.tile_pool(name="ps_score", bufs=2, space="PSUM") as ps_score, \
             tc.tile_pool(name="ps_av", bufs=2, space="PSUM") as ps_av:

            # weight DMAs first so nothing queues ahead of them
            wv = wqkv_d.rearrange("(f p) m -> p f m", p=128)
            w_sb = cpool.tile([128, FT, 384], BF16, tag="w")
            nc.sync.dma_start(w_sb[:, 0:4, :], wv[:, 0:4, :])
            nc.sync.dma_start(w_sb[:, 4:8, :], wv[:, 4:8, :])
            wout_sb = cpool.tile([128, D], BF16, tag="wout")
            nc.gpsimd.dma_start(wout_sb[:], wout_d)
            ident = cpool.tile([128, 128], BF16, tag="ident")
            masks.make_identity(nc, ident[:])

            # HAM warmup: dense matmuls on the identity while the input
            # DMAs land, so P1 starts at K=8/8 instead of half clock.
            for _ in range(40):
                pw = ps_score.tile([128, 1024], F32, tag="score", name="warm")
                nc.tensor.matmul(pw[:, 0:128], ident[:], ident[:],
                                 start=True, stop=True)

            # per-batch live tiles
            qT_t, kT_t, v_t, ostk_t, xt_t = {}, {}, {}, {}, {}

            def p1_load(b, tt):
                """Prefetch the xT chunk for token tile tt of batch b."""
                tok = slice(b * N + tt * 512, b * N + (tt + 1) * 512)
                xt_all = xt_pool.tile([128, FT, 512], BF16, tag="xt",
                                      name=f"xt{b}_{tt}")
                nc.sync.dma_start(xt_all[:], xT_v[:, :, tok].rearrange(
                    "f p n -> p f n"))
                xt_t[(b, tt)] = xt_all

            def p1_items(b, tt, blks=(0, 1, 2)):
                """Yield fine-grained work items (each ~1 matmul) for token
                tile tt of batch b: projection blocks (q=0,k=1,v=2) of 8
                accumulating matmuls + evac, then (if v included) 4
                transposes + evacs."""
                if tt == 0 and 0 in blks:
                    def setup():
                        qT_t[b] = qkv_pool.tile([128, N], BF16, tag="qT",
                                                name=f"qT{b}")
                        kT_t[b] = qkv_pool.tile([128, N], BF16, tag="kT",
                                                name=f"kT{b}")
                        v_t[b] = qkv_pool.tile([128, NJ, VW], BF16, tag="v",
                                               name=f"v{b}")
                        nc.vector.memset(v_t[b][:, :, DH:DH + 1], 1.0)
                        nc.vector.memset(v_t[b][:, :, 129:130], 1.0)
                    yield setup
                ts_ = slice(tt * 512, (tt + 1) * 512)
                state = {}

                def start_block(blk):
                    state['pp'] = ps_work.tile([128, 512], F32, tag="work", name="pp")
                    state['blk'] = blk

                def mm(ft):
                    blk = state['blk']
                    xt_all = xt_t[(b, tt)]
                    nc.tensor.matmul(
                        state['pp'][:],
                        w_sb[:, ft, blk * 128:(blk + 1) * 128],
                        xt_all[:, ft, :],
                        start=(ft == 0), stop=(ft == FT - 1))

                def evac(blk):
                    if blk == 2:
                        state['vts'] = vt_pool.tile([128, 512], BF16, tag="vt", name="vts")
                        nc.vector.tensor_copy(state['vts'][:], state['pp'][:])
                    elif blk == 0:
                        nc.vector.tensor_copy(qT_t[b][:, ts_], state['pp'][:])
                    else:
                        nc.vector.tensor_copy(kT_t[b][:, ts_], state['pp'][:])

                for blk in blks:  # subset of (q=0, k=1, v=2)
                    for ft in range(FT):
                        if ft == 0:
                            yield (lambda blk=blk, ft=ft: (start_block(blk),
                                                           mm(ft)))
                        else:
                            yield (lambda ft=ft: mm(ft))
                    yield (lambda blk=blk: evac(blk))
                if 2 not in blks:
                    # q-only pass is the last user of this xt tile
                    if 0 in blks:
                        yield (lambda: xt_t.pop((b, tt)) and None)
                    return

                def trans(sub):
                    pv = ps_work.tile([128, 512], F32, tag="work")
                    nc.tensor.matmul(
                        pv[:, 0:128], state['vts'][:, sub * 128:(sub + 1) * 128],
                        ident[:], start=True, stop=True)
                    ch = tt * 4 + sub
                    # [vA | 1 | vB | 1]: one strided copy lands vA at cols
                    # 0:64 and vB at cols 65:129 (stride-65 halves)
                    dst = v_t[b][:, ch, 0:130].rearrange(
                        "p (h w) -> p h w", h=2)[:, :, 0:DH]
                    src = pv[:, 0:128].rearrange("p (h w) -> p h w", h=2)
                    nc.vector.tensor_copy(dst, src)

                for sub in range(4):
                    yield (lambda sub=sub: trans(sub))
                if 0 in blks:
                    yield (lambda: xt_t.pop((b, tt)) and None)

            def p3_items(b, g, act_assist=False):
                """Output projection for token chunks 4g..4g+3 of batch b,
                as 8 matmul items + a store."""
                state = {}

                def setup():
                    state['fo'] = fout_pool.tile([128, 4, D], BF16, tag="fout", name="fo")

                def mm(ch, half):
                    tc_ = 4 * g + ch
                    pf = ps_work.tile([128, 512], F32, tag="work")
                    nc.tensor.matmul(
                        pf[:], ostk_t[b][:, tc_ * 128:(tc_ + 1) * 128],
                        wout_sb[:, half * 512:(half + 1) * 512],
                        start=True, stop=True)
                    dst = state['fo'][:, ch, half * 512:(half + 1) * 512]
                    if act_assist and half == 1:
                        nc.scalar.copy(dst, pf[:])
                    else:
                        nc.vector.tensor_copy(dst, pf[:])

                yield setup
                for ch in range(4):
                    for half in range(2):
                        yield (lambda ch=ch, half=half: mm(ch, half))

                def store():
                    base = b * N + 4 * g * 128
                    nc.gpsimd.dma_start(
                        out_d[base:base + 512, :].rearrange(
                            "(c p) m -> p c m", p=128),
                        state['fo'][:])
                yield store

            def p2(b, ni, dribble):
                """Attention for n_i tile ni of batch b; dribble is an
                iterator of side-work items, ~2 consumed per nj step."""
                if ni == 0:
                    ostk_t[b] = ostk_pool.tile([128, N], BF16, tag="ostk",
                                               name=f"ostk{b}")
                qT, kT, v_sb, ostk = qT_t[b], kT_t[b], v_t[b], ostk_t[b]
                pavA = ps_av.tile([128, 512], F32, tag="av")
                pavB = ps_av.tile([128, 512], F32, tag="av")
                qcol = slice(ni * 512, (ni + 1) * 512)
                at_q = []

                def scores(nj):
                    ps = ps_score.tile([128, 1024], F32, tag="score",
                                       name="ps")
                    kcol = slice(nj * 128, (nj + 1) * 128)
                    nc.tensor.matmul(ps[:, 0:512], kT[0:DH, kcol],
                                     qT[0:DH, qcol], start=True, stop=True)
                    nc.tensor.matmul(ps[:, 512:1024], kT[DH:128, kcol],
                                     qT[DH:128, qcol], start=True, stop=True)
                    at = attn_pool.tile([128, 1024], BF16, tag="attn",
                                        name="at")
                    nc.scalar.activation(at[:], ps[:], EXP, scale=0.125)
                    at_q.append((nj, at))

                def av():
                    nj, at = at_q.pop(0)
                    nc.tensor.matmul(
                        pavA[0:DH + 1, :], v_sb[:, nj, 0:DH + 1],
                        at[:, 0:512],
                        start=(nj == 0), stop=(nj == NJ - 1))
                    nc.tensor.matmul(
                        pavB[0:DH + 1, :], v_sb[:, nj, DH + 1:2 * DH + 2],
                        at[:, 512:1024],
                        start=(nj == 0), stop=(nj == NJ - 1))

                def drib(k):
                    for _ in range(k):
                        it = next(dribble, None)
                        if it is None:
                            return
                        it()

                # staggered: run scores/exp two iterations ahead of av, so
                # the first av of this ni issues well after the previous
                # ni's pav banks are released (no PE FIFO stall).
                scores(0)
                drib(2)
                scores(1)
                drib(2)
                for nj in range(2, NJ):
                    scores(nj)
                    av()
                    drib(3)
                av()
                drib(2)
                av()
                # evacuate accumulators fast (frees the av banks), then
                # normalize off the critical path. pav rows: 0:64 = av,
                # row 64 = softmax sums (ones column rides last).
                ocols = slice(ni * 512, (ni + 1) * 512)
                ovA = ov_pool.tile([DH, 512], F32, tag="ov")
                nc.vector.tensor_copy(ovA[:], pavA[0:DH, :])
                ovB = ov_pool.tile([DH, 512], F32, tag="ov")
                nc.vector.tensor_copy(ovB[:], pavB[0:DH, :])
                srow = smol_pool.tile([1, 1024], F32, tag="srow")
                nc.vector.tensor_copy(srow[0:1, 0:512], pavA[DH:DH + 1, :])
                nc.vector.tensor_copy(srow[0:1, 512:1024], pavB[DH:DH + 1, :])
                rcp = smol_pool.tile([1, 1024], F32, tag="rcp")
                nc.vector.reciprocal_approx_fast(rcp[:], srow[:])
                rbA = smol_pool.tile([DH, 512], F32, tag="rbA")
                nc.gpsimd.partition_broadcast(rbA[:], rcp[0:1, 0:512])
                rbB = smol_pool.tile([DH, 512], F32, tag="rbB")
                nc.gpsimd.partition_broadcast(rbB[:], rcp[0:1, 512:1024])
                nc.vector.tensor_mul(ostk[0:DH, ocols], rbA[:], ovA[:])
                nc.vector.tensor_mul(ostk[DH:128, ocols], rbB[:], ovB[:])

            def chain(*its):
                for it in its:
                    yield from it

            # software pipeline with fine-grained dribble:
            #   P2(b, ni) interleaves P1(b+1, tt=ni) and P3(b-1, g=ni)
            p1_load(0, 0)
            p1_load(0, 1)
            for it in chain(p1_items(0, 0), p1_items(0, 1)):
                it()
            p1_load(0, 2)
            p1_load(0, 3)
            for it in chain(p1_items(0, 2), p1_items(0, 3)):
                it()
            for b in range(B):
                for i in range(NI):
                    if b + 1 < B:
                        p1_load(b + 1, i)
                    items = []
                    if b + 1 < B:
                        items.append(p1_items(b + 1, i))
                    if b >= 1:
                        items.append(p3_items(b - 1, i))
                    if b == B - 1 and i >= 1:
                        items.append(p3_items(b, i - 1))
                    p2(b, i, chain(*items))
            for it in p3_items(B - 1, NI - 1, act_assist=True):
                it()

    nc.compile()
    return nc


def make_in_maps(x, w_qkv, w_out):
    xT_bf = np.ascontiguousarray(x.reshape(NT, D).T).astype(ml_dtypes.bfloat16)
    in_maps = []
    for c in range(8):
        cols = slice(c * 128, (c + 1) * 128)
        w_local = np.concatenate(
            [w_qkv[:, o * HEADS * DH:][:, cols] for o in range(3)], axis=1)
        in_maps.append({
            "xT": xT_bf,
            "wqkv": np.ascontiguousarray(w_local).astype(ml_dtypes.bfloat16),
            "wout": np.ascontiguousarray(w_out[c * 128:(c + 1) * 128, :]).astype(
                ml_dtypes.bfloat16),
        })
    return in_maps


def kernel(x, w_qkv, w_out):
    x = np.asarray(x, dtype=np.float32)
    w_qkv = np.asarray(w_qkv, dtype=np.float32)
    w_out = np.asarray(w_out, dtype=np.float32)
    if "nc" not in _CACHE:
        _CACHE["nc"] = build()
    nc = _CACHE["nc"]

    res = run_bass_kernel_spmd(nc, make_in_maps(x, w_qkv, w_out),
                               core_ids=list(range(8)))
    total = np.zeros((NT, D), dtype=np.float32)
    for c in range(8):
        total += np.asarray(res.results[c]["out"], dtype=np.float32)
    return total.reshape(B, N, D).astype(np.float32)


# revision 22
# speedup vs baseline: 1.0312x; 1.0312x over previous
"""Multi-head attention block (dense transformer) on 8 Trainium2 NeuronCores.

Problem: x [4, 2048, 1024] f32, w_qkv [1024, 3072], w_out [1024, 1024].
  qkv = x @ w_qkv -> split (3, 16 heads, 64) -> softmax(q k^T / 8) v -> @ w_out

Sharding: tensor-parallel over heads. Core c owns heads (2c, 2c+1):
  - w_qkv columns for q/k/v of those heads -> [1024, 384]
  - w_out rows for those heads            -> [128, 1024]
  - x is pre-transposed/cast on the host to xT [1024, 8192] bf16
  - each core computes a full-shape partial output [8192, 1024] bf16; the
    host f32 sum of the 8 partials is the all-reduce.

Per-core engine budget (measured): ACT exp of the 8 local attention
matrices = 256 x [128,1024] tiles ~ 285us; PE matmul work ~ 360us (the
binding resource); DVE evacuations ~ 225us. The emission is a
fine-grained weave: every nj iteration of P2 carries its score pair, exp,
av pair, AND a ~3-item dribble of P1 (next batch's qkv projection +
v transposes) and P3 (previous batch's output projection). The Tile
scheduler reorders by readiness+priority, but emission order sets both
the priority hints and the pool-buffer lifetimes, so the weave keeps ACT
fed while P1/P3 fill the PE gaps. P2 of each ni runs scores/exp two
iterations ahead of av so the previous ni's av-bank release (norm chain
on DVE) never blocks the PE. A 40-matmul identity warmup spins the PE
during the head DMA wait so the HAM clock gate opens before P1.

Layouts:
  qT/kT [128, 2048] bf16: rows = [head A d 0:64 | head B d 64:128].
  scoresT pair: row-group packed (K=64 concurrent matmuls h0/h64).
  v_sb [128, nj, 136]: [1 | vA(64) | vB(64) | 1 | pad] - one contiguous
    [128,128] copy from the PE transpose; ones columns give softmax sums
    as row 0 (head A, ones-first) / row 64 (head B, ones-last) of the
    M=65 av matmuls.
  PSUM: work 2 banks (P1/P3/transpose ping-pong), scores 2x2 banks
    (ping-pong against ACT), av 2 banks (pavA/pavB per ni).
"""

import numpy as np
import ml_dtypes

import concourse.bacc as bacc
import concourse.tile as tile
from concourse import mybir, masks
from concourse.bass_utils import run_bass_kernel_spmd

F32 = mybir.dt.float32
BF16 = mybir.dt.bfloat16
EXP = mybir.ActivationFunctionType.Exp

B = 4
PRIO = 1000000
N = 2048
D = 1024
HEADS = 16
DH = 64
NT = B * N           # 8192 tokens
FT = D // 128        # 8 feature chunks
NI = 4               # n_i tiles of 512 per batch
NJ = 16              # n_j chunks of 128 per batch
VW = 136             # v chunk: [1 | vA(64) | vB(64) | 1 | pad6] (16B-aligned)

_CACHE = {}


def build():
    nc = bacc.Bacc("TRN2", target_bir_lowering=False, debug=False, num_devices=1)
    xT_d = nc.dram_tensor("xT", [D, NT], BF16, kind="ExternalInput").ap()
    wqkv_d = nc.dram_tensor("wqkv", [D, 384], BF16, kind="ExternalInput").ap()
    wout_d = nc.dram_tensor("wout", [128, D], BF16, kind="ExternalInput").ap()
    out_d = nc.dram_tensor("out", [NT, D], BF16, kind="ExternalOutput").ap()
    xT_v = xT_d.rearrange("(f p) n -> f p n", p=128)

    with tile.TileContext(nc) as tc:
        with tc.tile_pool(name="const", bufs=1) as cpool, \
             tc.tile_pool(name="xt", bufs=4) as xt_pool, \
             tc.tile_pool(name="qkv", bufs=2) as qkv_pool, \
             tc.tile_pool(name="vt", bufs=3) as vt_pool, \
             tc.tile_pool(name="attn", bufs=4) as attn_pool, \
             tc.tile_pool(name="ostk", bufs=2) as ostk_pool, \
             tc.tile_pool(name="ov", bufs=4) as ov_pool, \
             tc.tile_pool(name="smol", bufs=6) as smol_pool, \
             tc.tile_pool(name="fout", bufs=2) as fout_pool, \
             tc.tile_pool(name="ps_work", bufs=2, space="PSUM") as ps_work, \
             tc.tile_pool(name="ps_score", bufs=2, space="PSUM") as ps_score, \
             tc.tile_pool(name="ps_av", bufs=2, space="PSUM") as ps_av:

            # weight DMAs first so nothing queues ahead of them
            wv = wqkv_d.rearrange("(f p) m -> p f m", p=128)
            w_sb = cpool.tile([128, FT, 384], BF16, tag="w")
            nc.sync.dma_start(w_sb[:, 0:4, :], wv[:, 0:4, :])
            nc.sync.dma_start(w_sb[:, 4:8, :], wv[:, 4:8, :])
            wout_sb = cpool.tile([128, D], BF16, tag="wout")
            nc.gpsimd.dma_start(wout_sb[:], wout_d)
            ident = cpool.tile([128, 128], BF16, tag="ident")
            masks.make_identity(nc, ident[:])

            # HAM warmup: dense matmuls on the identity while the input
            # DMAs land, so P1 starts at K=8/8 instead of half clock.
            for _ in range(64):
                pw = ps_score.tile([128, 1024], F32, tag="score", name="warm")
                nc.tensor.matmul(pw[:, 0:128], ident[:], ident[:],
                                 start=True, stop=True)

            # per-batch live tiles
            qT_t, kT_t, v_t, ostk_t, xt_t = {}, {}, {}, {}, {}

            def p1_load(b, tt):
                """Prefetch the xT chunk for token tile tt of batch b."""
                tok = slice(b * N + tt * 512, b * N + (tt + 1) * 512)
                xt_all = xt_pool.tile([128, FT, 512], BF16, tag="xt",
                                      name=f"xt{b}_{tt}")
                nc.sync.dma_start(xt_all[:], xT_v[:, :, tok].rearrange(
                    "f p n -> p f n"))
                xt_t[(b, tt)] = xt_all

            def p1_items(b, tt, blks=(0, 1, 2)):
                """Yield fine-grained work items (each ~1 matmul) for token
                tile tt of batch b: projection blocks (q=0,k=1,v=2) of 8
                accumulating matmuls + evac, then (if v included) 4
                transposes + evacs."""
                if tt == 0 and 0 in blks:
                    def setup():
                        qT_t[b] = qkv_pool.tile([128, N], BF16, tag="qT",
                                                name=f"qT{b}")
                        kT_t[b] = qkv_pool.tile([128, N], BF16, tag="kT",
                                                name=f"kT{b}")
                        v_t[b] = qkv_pool.tile([128, NJ, VW], BF16, tag="v",
                                               name=f"v{b}")
                        nc.vector.memset(v_t[b][:, :, DH:DH + 1], 1.0)
                        nc.vector.memset(v_t[b][:, :, 129:130], 1.0)
                    yield setup
                ts_ = slice(tt * 512, (tt + 1) * 512)
                state = {}

                def start_block(blk):
                    state['pp'] = ps_work.tile([128, 512], F32, tag="work", name="pp")
                    state['blk'] = blk

                def mm(ft):
                    blk = state['blk']
                    xt_all = xt_t[(b, tt)]
                    nc.tensor.matmul(
                        state['pp'][:],
                        w_sb[:, ft, blk * 128:(blk + 1) * 128],
                        xt_all[:, ft, :],
                        start=(ft == 0), stop=(ft == FT - 1))

                def evac(blk):
                    if blk == 2:
                        state['vts'] = vt_pool.tile([128, 512], BF16, tag="vt", name="vts")
                        nc.vector.tensor_copy(state['vts'][:], state['pp'][:])
                    elif blk == 0:
                        nc.vector.tensor_copy(qT_t[b][:, ts_], state['pp'][:])
                    else:
                        nc.vector.tensor_copy(kT_t[b][:, ts_], state['pp'][:])

                for blk in blks:  # subset of (q=0, k=1, v=2)
                    for ft in range(FT):
                        if ft == 0:
                            yield (lambda blk=blk, ft=ft: (start_block(blk),
                                                           mm(ft)))
                        else:
                            yield (lambda ft=ft: mm(ft))
                    yield (lambda blk=blk: evac(blk))
                if 2 not in blks:
                    # q-only pass is the last user of this xt tile
                    if 0 in blks:
                        yield (lambda: xt_t.pop((b, tt)) and None)
                    return

                def trans(sub):
                    pv = ps_work.tile([128, 512], F32, tag="work")
                    nc.tensor.matmul(
                        pv[:, 0:128], state['vts'][:, sub * 128:(sub + 1) * 128],
                        ident[:], start=True, stop=True)
                    ch = tt * 4 + sub
                    # [vA | 1 | vB | 1]: one strided copy lands vA at cols
                    # 0:64 and vB at cols 65:129 (stride-65 halves)
                    dst = v_t[b][:, ch, 0:130].rearrange(
                        "p (h w) -> p h w", h=2)[:, :, 0:DH]
                    src = pv[:, 0:128].rearrange("p (h w) -> p h w", h=2)
                    nc.vector.tensor_copy(dst, src)

                for sub in range(4):
                    yield (lambda sub=sub: trans(sub))
                if 0 in blks:
                    yield (lambda: xt_t.pop((b, tt)) and None)

            def p3_items(b, g, act_assist=False, split_store=False):
                """Output projection for token chunks 4g..4g+3 of batch b,
                as 8 matmul items + store(s)."""
                state = {}

                def setup():
                    state['fo'] = fout_pool.tile([128, 4, D], BF16, tag="fout", name="fo")

                def mm(ch, half):
                    tc_ = 4 * g + ch
                    pf = ps_work.tile([128, 512], F32, tag="work")
                    nc.tensor.matmul(
                        pf[:], ostk_t[b][:, tc_ * 128:(tc_ + 1) * 128],
                        wout_sb[:, half * 512:(half + 1) * 512],
                        start=True, stop=True)
                    dst = state['fo'][:, ch, half * 512:(half + 1) * 512]
                    if act_assist and half == 1:
                        nc.scalar.copy(dst, pf[:])
                    else:
                        nc.vector.tensor_copy(dst, pf[:])

                def store(c0, c1):
                    base = b * N + (4 * g + c0) * 128
                    nc.gpsimd.dma_start(
                        out_d[base:base + (c1 - c0) * 128, :].rearrange(
                            "(c p) m -> p c m", p=128),
                        state['fo'][:, c0:c1, :])

                yield setup
                for ch in range(4):
                    for half in range(2):
                        yield (lambda ch=ch, half=half: mm(ch, half))
                    if split_store and ch % 2 == 1:
                        yield (lambda c0=ch - 1, c1=ch + 1: store(c0, c1))
                if not split_store:
                    yield (lambda: store(0, 4))

            def p2(b, ni, dribble):
                """Attention for n_i tile ni of batch b; dribble is an
                iterator of side-work items, ~2 consumed per nj step."""
                if ni == 0:
                    ostk_t[b] = ostk_pool.tile([128, N], BF16, tag="ostk",
                                               name=f"ostk{b}")
                qT, kT, v_sb, ostk = qT_t[b], kT_t[b], v_t[b], ostk_t[b]
                pavA = ps_av.tile([128, 512], F32, tag="av")
                pavB = ps_av.tile([128, 512], F32, tag="av")
                qcol = slice(ni * 512, (ni + 1) * 512)
                at_q = []

                def scores(nj):
                    ps = ps_score.tile([128, 1024], F32, tag="score",
                                       name="ps")
                    kcol = slice(nj * 128, (nj + 1) * 128)
                    with tc.high_priority(offset=400):
                        nc.tensor.matmul(ps[:, 0:512], kT[0:DH, kcol],
                                         qT[0:DH, qcol], start=True,
                                         stop=True)
                        nc.tensor.matmul(ps[:, 512:1024], kT[DH:128, kcol],
                                         qT[DH:128, qcol], start=True,
                                         stop=True)
                    at = attn_pool.tile([128, 1024], BF16, tag="attn",
                                        name="at")
                    nc.scalar.activation(at[:], ps[:], EXP, scale=0.125)
                    at_q.append((nj, at))

                def av():
                    nj, at = at_q.pop(0)
                    nc.tensor.matmul(
                        pavA[0:DH + 1, :], v_sb[:, nj, 0:DH + 1],
                        at[:, 0:512],
                        start=(nj == 0), stop=(nj == NJ - 1))
                    nc.tensor.matmul(
                        pavB[0:DH + 1, :], v_sb[:, nj, DH + 1:2 * DH + 2],
                        at[:, 512:1024],
                        start=(nj == 0), stop=(nj == NJ - 1))

                def drib(k):
                    for _ in range(k):
                        it = next(dribble, None)
                        if it is None:
                            return
                        it()

                # staggered: run scores/exp two iterations ahead of av, so
                # the first av of this ni issues well after the previous
                # ni's pav banks are released (no PE FIFO stall).
                scores(0)
                drib(2)
                scores(1)
                drib(2)
                for nj in range(2, NJ):
                    scores(nj)
                    av()
                    drib(3)
                av()
                drib(2)
                av()
                # evacuate accumulators fast (frees the av banks), then
                # normalize off the critical path. pav rows: 0:64 = av,
                # row 64 = softmax sums (ones column rides last).
                ocols = slice(ni * 512, (ni + 1) * 512)
                ovA = ov_pool.tile([DH, 512], F32, tag="ov")
                nc.vector.tensor_copy(ovA[:], pavA[0:DH, :])
                ovB = ov_pool.tile([DH, 512], F32, tag="ov")
                nc.vector.tensor_copy(ovB[:], pavB[0:DH, :])
                srow = smol_pool.tile([1, 1024], F32, tag="srow")
                nc.vector.tensor_copy(srow[0:1, 0:512], pavA[DH:DH + 1, :])
                nc.vector.tensor_copy(srow[0:1, 512:1024], pavB[DH:DH + 1, :])
                rcp = smol_pool.tile([1, 1024], F32, tag="rcp")
                nc.vector.reciprocal_approx_fast(rcp[:], srow[:])
                rbA = smol_pool.tile([DH, 512], F32, tag="rbA")
                nc.gpsimd.partition_broadcast(rbA[:], rcp[0:1, 0:512])
                rbB = smol_pool.tile([DH, 512], F32, tag="rbB")
                nc.gpsimd.partition_broadcast(rbB[:], rcp[0:1, 512:1024])
                nc.vector.tensor_mul(ostk[0:DH, ocols], rbA[:], ovA[:])
                nc.vector.tensor_mul(ostk[DH:128, ocols], rbB[:], ovB[:])

            def chain(*its):
                for it in its:
                    yield from it

            # software pipeline with fine-grained dribble:
            #   P2(b, ni) interleaves P1(b+1, tt=ni) and P3(b-1, g=ni)
            p1_load(0, 0)
            p1_load(0, 1)
            for it in chain(p1_items(0, 0), p1_items(0, 1)):
                it()
            p1_load(0, 2)
            p1_load(0, 3)
            for it in chain(p1_items(0, 2), p1_items(0, 3)):
                it()
            for b in range(B):
                for i in range(NI):
                    if b + 1 < B:
                        p1_load(b + 1, i)
                    items = []
                    if b + 1 < B:
                        items.append(p1_items(b + 1, i))
                    if b >= 1:
                        items.append(p3_items(b - 1, i))
                    if b == B - 1 and i >= 1:
                        items.append(p3_items(b, i - 1))
                    p2(b, i, chain(*items))
            for it in p3_items(B - 1, NI - 1, act_assist=True,
                               split_store=True):
                it()

    nc.compile()
    return nc


def make_in_maps(x, w_qkv, w_out):
    xT_bf = np.ascontiguousarray(x.reshape(NT, D).T).astype(ml_dtypes.bfloat16)
    in_maps = []
    for c in range(8):
        cols = slice(c * 128, (c + 1) * 128)
        w_local = np.concatenate(
            [w_qkv[:, o * HEADS * DH:][:, cols] for o in range(3)], axis=1)
        in_maps.append({
            "xT": xT_bf,
            "wqkv": np.ascontiguousarray(w_local).astype(ml_dtypes.bfloat16),
            "wout": np.ascontiguousarray(w_out[c * 128:(c + 1) * 128, :]).astype(
                ml_dtypes.bfloat16),
        })
    return in_maps


def kernel(x, w_qkv, w_out):
    x = np.asarray(x, dtype=np.float32)
    w_qkv = np.asarray(w_qkv, dtype=np.float32)
    w_out = np.asarray(w_out, dtype=np.float32)
    if "nc" not in _CACHE:
        _CACHE["nc"] = build()
    nc = _CACHE["nc"]

    res = run_bass_kernel_spmd(nc, make_in_maps(x, w_qkv, w_out),
                               core_ids=list(range(8)))
    total = np.zeros((NT, D), dtype=np.float32)
    for c in range(8):
        total += np.asarray(res.results[c]["out"], dtype=np.float32)
    return total.reshape(B, N, D).astype(np.float32)
